# revision 5
# baseline (speedup 1.0000x reference)
"""LBLHighwayBiLm Trainium2 kernel v3 (8-core data-parallel over batch).

Channel-major fp16. Per layer, one loop over 4 token-groups (tg = one batch
row of 1024 tokens); within each tg both directions' conv chunk, sublayer-A
chunk, sublayer-B chunk are emitted back-to-back so the PE queue always holds
ready matmuls while ACT/DVE drain PSUM. Conv: 4 taps as accumulating
identity matmuls on PE ([128,512] PSUM chunks, 1 bank, bufs=2) + tap 4 fused
into the DVE STT eviction. Highway matmuls: [128,1024] PSUM tiles (2 banks,
bufs=3); sigmoid/relu (+bias) evictions on ACT; combine t/u/x1 TTs on
DVE/Pool per knob. Output DMA per chunk.
"""

import numpy as np

import concourse.bacc as bacc
import concourse.tile as tile
import concourse.mybir as mybir
from concourse.bass_utils import run_bass_kernel_spmd

F16 = mybir.dt.float16
F32 = mybir.dt.float32
AOP = mybir.AluOpType
AFT = mybir.ActivationFunctionType

N_LAYERS = 2
N_HW = 2
W = 4
D = 256
B, S = 32, 1024
NCORES = 8
BLOC = B // NCORES
T = BLOC * S
PB = D // 128
EB = (2 * D) // 128
ROW0 = S + 2 * W            # 1032
ROW1 = S + W                # 1028
CH = 1024

# --- knobs -------------------------------------------------------------------
N_PE_TAPS = 4               # taps 0..N-1 on PE; tap W fused in STT eviction
CONV_CH = 512               # conv PSUM chunk (1 bank)
# conv implementation per (l, di):
#   "pe"       4 taps PE, tap4 fused in DVE STT eviction
#   "pe3_pool" 3 taps PE, Pool joins taps 3+4 in SBUF, DVE STT adds PSUM
#   "pe4_act"  4 taps PE, ACT copy-evicts PSUM, Pool STT adds tap4
#   "dve"      all-DVE tensor_scalar/tensor_tensor tree
CONV_IMPL = {}
# combine engine per (l, di, op) with ops 't','u','x1'; default 'v'
COMBINE_ENG = {}
ENG_DEFAULT = {"t": "v", "u": "v", "x1": "v"}
CONV_EVICT_ENG = "v"        # conv PSUM eviction STT; GPSIMD can't read PSUM
MM_BUFS = 3
CV_BUFS = 2
SCRATCH_BUFS = 4
RELU_DVE_LAST = False       # last token-group relu evictions on DVE
PE_WARMUP = 4               # dummy 512-row matmuls at t~0 to ramp PE p-state


def _eng(nc, code):
    return {"v": nc.vector, "p": nc.gpsimd}[code]


def build_bass(params):
    nc = bacc.Bacc(target_bir_lowering=False)

    x_in = nc.dram_tensor("x", [PB, 128, BLOC * ROW0], F16, kind="ExternalInput")
    out = nc.dram_tensor(
        "out", [N_LAYERS, 2, PB, 128, T], F16, kind="ExternalOutput"
    )

    wt_dram = nc.inline_tensor(params["wt"], name="wt")
    bias_dram = nc.inline_tensor(params["bias"], name="bias")
    pad_dram = nc.inline_tensor(params["pad1"], name="pad1")
    fw = params["fwd_w"]
    bw = params["bwd_w"]

    with tile.TileContext(nc) as tc:
        consts = tc.alloc_tile_pool(name="consts", bufs=1)
        bufs = tc.alloc_tile_pool(name="bufs", bufs=1)
        scratch = tc.alloc_tile_pool(name="scratch", bufs=SCRATCH_BUFS)
        psum_mm = tc.alloc_tile_pool(name="psum_mm", bufs=MM_BUFS, space="PSUM")
        psum_cv = tc.alloc_tile_pool(name="psum_cv", bufs=CV_BUFS, space="PSUM")

        # ---- input + constants, ordered for startup: row-0 first, then the
        # identity taps, then remaining rows interleaved with weights --------
        xpad0 = [bufs.tile([128, BLOC * ROW0], F16, name=f"xpad0_{blk}")
                 for blk in range(PB)]

        def load_row(r):
            for blk in range(PB):
                nc.gpsimd.dma_start(
                    out=xpad0[blk][:, r * ROW0:(r + 1) * ROW0],
                    in_=x_in[blk, :, r * ROW0:(r + 1) * ROW0],
                )

        if PE_WARMUP:
            wup = bufs.tile([128, 512], F16, name="wup")
            nc.gpsimd.memset(wup, 0.0)
            wp = psum_cv.tile([128, 512], F32, tag="cv", name="wup_ps")
            for i in range(PE_WARMUP):
                nc.tensor.matmul(wp, lhsT=wup[:, :128], rhs=wup,
                                 start=True, stop=True)

        load_row(0)
        # identities built on-chip (no DMA latency at startup)
        from concourse import masks as _masks
        id0 = consts.tile([128, 128], F16, name="id0")
        _masks.make_identity(nc, id0)
        id_sb = {}
        for l in range(N_LAYERS):
            for di in range(2):
                taps_ld = fw[l] if di == 0 else bw[l]
                for j in range(N_PE_TAPS):
                    it = consts.tile([128, 128], F16, name=f"id{l}{di}{j}")
                    nc.vector.tensor_scalar_mul(it, id0, float(taps_ld[j]))
                    id_sb[(l, di, j)] = it
        bias_sb = consts.tile([128, N_LAYERS * 2 * N_HW * EB], F32, name="bias_sb")
        nc.sync.dma_start(out=bias_sb, in_=bias_dram[:, :])
        pad_sb = consts.tile([128, 2 * PB * W], F16, name="pad_sb")
        nc.scalar.dma_start(out=pad_sb, in_=pad_dram[:, :])
        load_row(1)
        wt_sb = {}
        for l in range(N_LAYERS):
            for h in range(N_HW):
                for di in range(2):
                    for kb in range(PB):
                        wtt = consts.tile([128, 2 * D], F16,
                                          name=f"wt{l}{di}{h}{kb}")
                        eng = nc.sync if di == 0 else nc.scalar
                        eng.dma_start(out=wtt, in_=wt_dram[l, di, h, kb])
                        wt_sb[(l, di, h, kb)] = wtt

        def bias_ap(l, di, h, eb):
            i = ((l * 2 + di) * N_HW + h) * EB + eb
            return bias_sb[:, i:i + 1]

        # layer-1 padded buffers + halos
        xpadf = [bufs.tile([128, BLOC * ROW1], F16, name=f"xpf{blk}")
                 for blk in range(PB)]
        xpadb = [bufs.tile([128, BLOC * ROW1], F16, name=f"xpb{blk}")
                 for blk in range(PB)]
        for blk in range(PB):
            for r in range(BLOC):
                nc.vector.tensor_copy(
                    xpadf[blk][:, r * ROW1:r * ROW1 + W],
                    pad_sb[:, (0 * PB + blk) * W:(0 * PB + blk + 1) * W],
                )
                nc.vector.tensor_copy(
                    xpadb[blk][:, r * ROW1 + S:(r + 1) * ROW1],
                    pad_sb[:, (1 * PB + blk) * W:(1 * PB + blk + 1) * W],
                )

        # activation buffers
        f_t = {di: [bufs.tile([128, T], F16, tag=f"f{di}{blk}",
                              name=f"f{di}{blk}") for blk in range(PB)]
               for di in range(2)}
        xa_t = {di: [bufs.tile([128, T], F16, tag=f"xa{di}{blk}",
                               name=f"xa{di}{blk}") for blk in range(PB)]
                for di in range(2)}
        # layer-1 B outputs alias the (by then dead) conv-output tiles; the
        # per-chunk WAR order (B writes chunk tg after A read it) is tracked
        # by the tile framework.
        xb1 = f_t

        def conv_chunk_dve(l, di, r, dst, src, row_len, base_off, taps):
            """One batch-row conv chunk on DVE: 5 tensor_scalar + 4 adds."""
            for blk in range(PB):
                def sl(j):
                    o = r * row_len + base_off + j
                    return src[blk][:, o:o + CH]
                a = scratch.tile([128, CH], F16, tag="ca", name=f"ca{l}{di}{blk}{r}")
                b = scratch.tile([128, CH], F16, tag="cb", name=f"cb{l}{di}{blk}{r}")
                c = scratch.tile([128, CH], F16, tag="cc", name=f"cc{l}{di}{blk}{r}")
                nc.vector.tensor_scalar_mul(a, sl(0), float(taps[0]))
                nc.vector.tensor_scalar_mul(b, sl(1), float(taps[1]))
                nc.vector.tensor_tensor(a, a, b, AOP.add)
                nc.vector.tensor_scalar_mul(b, sl(2), float(taps[2]))
                nc.vector.tensor_scalar_mul(c, sl(3), float(taps[3]))
                nc.vector.tensor_tensor(b, b, c, AOP.add)
                nc.vector.tensor_scalar_mul(c, sl(4), float(taps[4]))
                nc.vector.tensor_tensor(a, a, b, AOP.add)
                acc = dst[blk][:, r * CH:(r + 1) * CH]
                nc.vector.tensor_tensor(acc, a, c, AOP.add)

        def conv_chunk(l, di, r, dst, src, row_len, base_off, taps):
            """One batch-row conv chunk: CONV_CH sub-chunks."""
            mode = CONV_IMPL.get((l, di), "pe")
            if mode == "dve":
                conv_chunk_dve(l, di, r, dst, src, row_len, base_off, taps)
                return
            n_pe = 3 if mode == "pe3_pool" else N_PE_TAPS
            for c0 in range(0, CH, CONV_CH):
                for blk in range(PB):
                    def sl(j):
                        o = r * row_len + base_off + j + c0
                        return src[blk][:, o:o + CONV_CH]
                    p = psum_cv.tile([128, CONV_CH], F32, tag="cv",
                                     name=f"cv{l}{di}{blk}{r}{c0}")
                    for half in range(CONV_CH // 512):
                        hs = slice(half * 512, (half + 1) * 512)
                        for j in range(n_pe):
                            nc.tensor.matmul(
                                p[:, hs],
                                lhsT=id_sb[(l, di, j)],
                                rhs=sl(j)[:, hs],
                                start=(j == 0),
                                stop=(j == n_pe - 1),
                            )
                    acc = dst[blk][:, r * CH + c0:r * CH + c0 + CONV_CH]
                    if mode == "pe":
                        nc.vector.scalar_tensor_tensor(
                            acc, sl(W), float(taps[W]), p, AOP.mult, AOP.add)
                    elif mode == "pe3_pool":
                        # Pool joins taps 3+4 in SBUF (TS+TS+TT; Pool lacks
                        # the STT opcode); DVE adds the PSUM part
                        tm = scratch.tile([128, CONV_CH], F16, tag="cm",
                                          name=f"cm{l}{di}{blk}{r}{c0}")
                        nc.gpsimd.tensor_scalar_mul(tm, sl(4), float(taps[4]))
                        tm2 = scratch.tile([128, CONV_CH], F16, tag="cn",
                                           name=f"cn{l}{di}{blk}{r}{c0}")
                        nc.gpsimd.tensor_scalar_mul(tm2, sl(3), float(taps[3]))
                        nc.gpsimd.tensor_tensor(tm2, tm2, tm, AOP.add)
                        nc.vector.scalar_tensor_tensor(
                            acc, tm2, 1.0, p, AOP.mult, AOP.add)
                    else:  # pe4_act
                        tm = scratch.tile([128, CONV_CH], F16, tag="cm",
                                          name=f"cm{l}{di}{blk}{r}{c0}")
                        nc.scalar.activation(tm, p, AFT.Copy, bias=0.0,
                                             scale=1.0)
                        tm2 = scratch.tile([128, CONV_CH], F16, tag="cn",
                                           name=f"cn{l}{di}{blk}{r}{c0}")
                        nc.gpsimd.tensor_scalar_mul(tm2, sl(W), float(taps[W]))
                        nc.gpsimd.tensor_tensor(acc, tm2, tm, AOP.add)

        def hw_chunk(l, di, h, tg, x0, x1, x1_row_len, x1_off, c0=0, cw=CH):
            """Token-columns [tg*CH+c0, +cw) of one highway sublayer."""
            gt = {}
            rt = {}
            for half_kind in range(2):      # 0 = nonlin, 1 = gate
                for blk in range(PB):
                    eb = half_kind * PB + blk
                    p = psum_mm.tile([128, cw], F32, tag="mm",
                                     name=f"mm{l}{di}{h}{eb}{tg}{c0}")
                    for h0 in range(0, cw, 512):
                        hw_ = min(512, cw - h0)
                        hs = slice(tg * CH + c0 + h0, tg * CH + c0 + h0 + hw_)
                        for kb in range(PB):
                            nc.tensor.matmul(
                                p[:, h0:h0 + hw_],
                                lhsT=wt_sb[(l, di, h, kb)][:, eb * 128:(eb + 1) * 128],
                                rhs=x0[kb][:, hs],
                                start=(kb == 0),
                                stop=(kb == PB - 1),
                            )
                    if half_kind == 0:
                        t_ = scratch.tile([128, cw], F16, tag="r",
                                          name=f"r{l}{di}{h}{blk}{tg}{c0}")
                        if RELU_DVE_LAST and l == N_LAYERS - 1 and tg == BLOC - 1:
                            nc.vector.tensor_scalar(
                                t_, p, bias_ap(l, di, h, blk), 0.0,
                                AOP.add, AOP.max)
                        else:
                            nc.scalar.activation(
                                t_, p, AFT.Relu, bias=bias_ap(l, di, h, blk),
                                scale=1.0)
                        rt[blk] = t_
                    else:
                        t_ = scratch.tile([128, cw], F16, tag="g",
                                          name=f"g{l}{di}{h}{blk}{tg}{c0}")
                        nc.scalar.activation(
                            t_, p, AFT.Sigmoid,
                            bias=bias_ap(l, di, h, PB + blk), scale=1.0)
                        gt[blk] = t_
            for blk in range(PB):
                x0c = x0[blk][:, tg * CH + c0:tg * CH + c0 + cw]
                tt = scratch.tile([128, cw], F16, tag="t",
                                  name=f"t{l}{di}{h}{blk}{tg}{c0}")
                _eng(nc, COMBINE_ENG.get((l, di, 't'), ENG_DEFAULT['t'])).tensor_tensor(
                    tt, x0c, rt[blk], AOP.subtract)
                ut = scratch.tile([128, cw], F16, tag="u",
                                  name=f"u{l}{di}{h}{blk}{tg}{c0}")
                _eng(nc, COMBINE_ENG.get((l, di, 'u'), ENG_DEFAULT['u'])).tensor_tensor(
                    ut, gt[blk], tt, AOP.mult)
                o = tg * x1_row_len + x1_off + c0
                x1c = x1[blk][:, o:o + cw]
                _eng(nc, COMBINE_ENG.get((l, di, 'x1'), ENG_DEFAULT['x1'])).tensor_tensor(
                    x1c, ut, rt[blk], AOP.add)

        # ---- the network: software-pipelined stage-wise emission ------------
        def src_of(l):
            if l == 0:
                return {0: (xpad0, ROW0, 0), 1: (xpad0, ROW0, W)}
            return {0: (xpadf, ROW1, 0), 1: (xpadb, ROW1, 0)}

        def b_dst_of(l):
            if l == 0:
                return {0: (xpadf, ROW1, W), 1: (xpadb, ROW1, 0)}
            return {0: (xb1[0], CH, 0), 1: (xb1[1], CH, 0)}

        def emit_conv(l, tg):
            for di in range(2):
                taps = fw[l] if di == 0 else bw[l]
                s_tiles, rl, off = src_of(l)[di]
                conv_chunk(l, di, tg, f_t[di], s_tiles, rl, off, taps)

        def emit_A(l, tg, split=False):
            for c0, cw in ([(0, 512), (512, 512)] if split else [(0, CH)]):
                for di in range(2):
                    hw_chunk(l, di, 0, tg, f_t[di], xa_t[di], CH, 0, c0, cw)

        def emit_B(l, tg, split=False):
            for c0, cw in ([(0, 512), (512, 512)] if split else [(0, CH)]):
                for di in range(2):
                    x1t, rl, off = b_dst_of(l)[di]
                    hw_chunk(l, di, 1, tg, xa_t[di], x1t, rl, off, c0, cw)
                    for blk in range(PB):
                        o = tg * rl + off + c0
                        nc.sync.dma_start(
                            out=out[l, di, blk][:, tg * CH + c0:tg * CH + c0 + cw],
                            in_=x1t[blk][:, o:o + cw],
                        )

        # conv chunks interleaved per token group act as PE filler while
        # ACT/DVE drain the highway PSUM groups. Rows 2,3 load lazily so the
        # Pool queue isn't clogged ahead of the first conv evictions.
        for l in range(N_LAYERS):
            for tg in range(BLOC):
                emit_conv(l, tg)
                if l == 0 and tg < 2:
                    load_row(tg + 2)
                last = (l == N_LAYERS - 1 and tg == BLOC - 1)
                emit_A(l, tg)
                emit_B(l, tg, split=last)

        psum_cv.release()
        psum_mm.release()
        scratch.release()
        bufs.release()
        consts.release()

    nc.finalize()
    return nc


def _prep_params(inputs):
    fwd_hw_W = np.asarray(inputs["fwd_hw_W"], np.float32)
    bwd_hw_W = np.asarray(inputs["bwd_hw_W"], np.float32)
    wt = np.empty((N_LAYERS, 2, N_HW, PB, 128, 2 * D), np.float32)
    for l in range(N_LAYERS):
        for di, Wsrc in ((0, fwd_hw_W), (1, bwd_hw_W)):
            for h in range(N_HW):
                wT = Wsrc[l, h].T
                wt[l, di, h] = wT.reshape(PB, 128, 2 * D)
    wt = wt.astype(np.float16)

    fwd_hw_b = np.asarray(inputs["fwd_hw_b"], np.float32)
    bwd_hw_b = np.asarray(inputs["bwd_hw_b"], np.float32)
    bias = np.empty((128, N_LAYERS * 2 * N_HW * EB), np.float32)
    for l in range(N_LAYERS):
        for di, bsrc in ((0, fwd_hw_b), (1, bwd_hw_b)):
            for h in range(N_HW):
                for eb in range(EB):
                    i = ((l * 2 + di) * N_HW + h) * EB + eb
                    bias[:, i] = bsrc[l, h, eb * 128:(eb + 1) * 128]

    fwd_pad = np.asarray(inputs["fwd_pad"], np.float32)
    bwd_pad = np.asarray(inputs["bwd_pad"], np.float32)
    pad1 = np.empty((128, 2 * PB * W), np.float32)
    for di, psrc in ((0, fwd_pad), (1, bwd_pad)):
        pT = psrc[1].T.reshape(PB, 128, W)
        for blk in range(PB):
            pad1[:, (di * PB + blk) * W:(di * PB + blk + 1) * W] = pT[blk]
    pad1 = pad1.astype(np.float16)

    fwd_w = np.asarray(inputs["fwd_w"], np.float32)
    bwd_w = np.asarray(inputs["bwd_w"], np.float32)

    return {
        "wt": np.ascontiguousarray(wt),
        "bias": np.ascontiguousarray(bias),
        "pad1": np.ascontiguousarray(pad1),
        "fwd_w": [[float(v) for v in row] for row in fwd_w],
        "bwd_w": [[float(v) for v in row] for row in bwd_w],
    }


def _prep_core_input(x_core, fwd_pad, bwd_pad):
    xt = np.ascontiguousarray(x_core.transpose(2, 0, 1))
    blocks = xt.reshape(PB, 128, BLOC, S)
    padded = np.empty((PB, 128, BLOC, ROW0), np.float32)
    padded[:, :, :, W:W + S] = blocks
    fr = fwd_pad[0].T.reshape(PB, 128, W)
    bk = bwd_pad[0].T.reshape(PB, 128, W)
    padded[:, :, :, :W] = fr[:, :, None, :]
    padded[:, :, :, W + S:] = bk[:, :, None, :]
    return np.ascontiguousarray(padded.reshape(PB, 128, BLOC * ROW0).astype(np.float16))


_NC_CACHE = {}


def kernel(**inputs):
    params = _prep_params(inputs)
    import hashlib
    h = hashlib.sha256()
    for k in ("wt", "bias", "pad1"):
        h.update(params[k].tobytes())
    h.update(repr(params["fwd_w"]).encode())
    h.update(repr(params["bwd_w"]).encode())
    key = h.hexdigest()
    if key not in _NC_CACHE:
        _NC_CACHE[key] = build_bass(params)
    nc = _NC_CACHE[key]

    x = np.asarray(inputs["inputs"], np.float32)
    fwd_pad = np.asarray(inputs["fwd_pad"], np.float32)
    bwd_pad = np.asarray(inputs["bwd_pad"], np.float32)
    in_maps = [
        {"x": _prep_core_input(x[c * BLOC:(c + 1) * BLOC], fwd_pad, bwd_pad)}
        for c in range(NCORES)
    ]
    res = run_bass_kernel_spmd(nc, in_maps, core_ids=list(range(NCORES)))

    y = np.empty((N_LAYERS, B, S, 2 * D), np.float32)
    for c in range(NCORES):
        o = np.asarray(res.results[c]["out"]).astype(np.float32)
        o = o.reshape(N_LAYERS, 2, PB, 128, BLOC, S)
        o = o.transpose(0, 4, 5, 1, 2, 3).reshape(N_LAYERS, BLOC, S, 2 * D)
        y[:, c * BLOC:(c + 1) * BLOC] = o
    return y


# revision 6
# speedup vs baseline: 1.0018x; 1.0018x over previous
"""LBLHighwayBiLm Trainium2 kernel v3 (8-core data-parallel over batch).

Channel-major fp16. Per layer, one loop over 4 token-groups (tg = one batch
row of 1024 tokens); within each tg both directions' conv chunk, sublayer-A
chunk, sublayer-B chunk are emitted back-to-back so the PE queue always holds
ready matmuls while ACT/DVE drain PSUM. Conv: 4 taps as accumulating
identity matmuls on PE ([128,512] PSUM chunks, 1 bank, bufs=2) + tap 4 fused
into the DVE STT eviction. Highway matmuls: [128,1024] PSUM tiles (2 banks,
bufs=3); sigmoid/relu (+bias) evictions on ACT; combine t/u/x1 TTs on
DVE/Pool per knob. Output DMA per chunk.
"""

import numpy as np

import concourse.bacc as bacc
import concourse.tile as tile
import concourse.mybir as mybir
from concourse.bass_utils import run_bass_kernel_spmd

F16 = mybir.dt.float16
F32 = mybir.dt.float32
AOP = mybir.AluOpType
AFT = mybir.ActivationFunctionType

N_LAYERS = 2
N_HW = 2
W = 4
D = 256
B, S = 32, 1024
NCORES = 8
BLOC = B // NCORES
T = BLOC * S
PB = D // 128
EB = (2 * D) // 128
ROW0 = S + 2 * W            # 1032
ROW1 = S + W                # 1028
CH = 1024

# --- knobs -------------------------------------------------------------------
N_PE_TAPS = 4               # taps 0..N-1 on PE; tap W fused in STT eviction
CONV_CH = 512               # conv PSUM chunk (1 bank)
# conv implementation per (l, di):
#   "pe"       4 taps PE, tap4 fused in DVE STT eviction
#   "pe3_pool" 3 taps PE, Pool joins taps 3+4 in SBUF, DVE STT adds PSUM
#   "pe4_act"  4 taps PE, ACT copy-evicts PSUM, Pool STT adds tap4
#   "dve"      all-DVE tensor_scalar/tensor_tensor tree
CONV_IMPL = {}
# combine engine per (l, di, op) with ops 't','u','x1'; default 'v'
COMBINE_ENG = {}
ENG_DEFAULT = {"t": "v", "u": "v", "x1": "v"}
CONV_EVICT_ENG = "v"        # conv PSUM eviction STT; GPSIMD can't read PSUM
MM_BUFS = 3
CV_BUFS = 2
SCRATCH_BUFS = 4
RELU_DVE_LAST = False       # last token-group relu evictions on DVE
PE_WARMUP = 4               # dummy 512-row matmuls at t~0 to ramp PE p-state


def _eng(nc, code):
    return {"v": nc.vector, "p": nc.gpsimd}[code]


def build_bass(params):
    nc = bacc.Bacc(target_bir_lowering=False)

    x_in = nc.dram_tensor("x", [PB, 128, BLOC * ROW0], F16, kind="ExternalInput")
    out = nc.dram_tensor(
        "out", [N_LAYERS, 2, PB, 128, T], F16, kind="ExternalOutput"
    )

    wt_dram = nc.inline_tensor(params["wt"], name="wt")
    bias_dram = nc.inline_tensor(params["bias"], name="bias")
    pad_dram = nc.inline_tensor(params["pad1"], name="pad1")
    fw = params["fwd_w"]
    bw = params["bwd_w"]

    with tile.TileContext(nc) as tc:
        consts = tc.alloc_tile_pool(name="consts", bufs=1)
        bufs = tc.alloc_tile_pool(name="bufs", bufs=1)
        scratch = tc.alloc_tile_pool(name="scratch", bufs=SCRATCH_BUFS)
        psum_mm = tc.alloc_tile_pool(name="psum_mm", bufs=MM_BUFS, space="PSUM")
        psum_cv = tc.alloc_tile_pool(name="psum_cv", bufs=CV_BUFS, space="PSUM")

        # ---- input + constants, ordered for startup: row-0 first, then the
        # identity taps, then remaining rows interleaved with weights --------
        xpad0 = [bufs.tile([128, BLOC * ROW0], F16, name=f"xpad0_{blk}")
                 for blk in range(PB)]

        def load_row(r):
            for blk in range(PB):
                nc.gpsimd.dma_start(
                    out=xpad0[blk][:, r * ROW0:(r + 1) * ROW0],
                    in_=x_in[blk, :, r * ROW0:(r + 1) * ROW0],
                )

        if PE_WARMUP:
            wup = bufs.tile([128, 512], F16, name="wup")
            nc.gpsimd.memset(wup, 0.0)
            wp = psum_cv.tile([128, 512], F32, tag="cv", name="wup_ps")
            for i in range(PE_WARMUP):
                nc.tensor.matmul(wp, lhsT=wup[:, :128], rhs=wup,
                                 start=True, stop=True)

        load_row(0)
        # identities built on-chip (no DMA latency at startup)
        from concourse import masks as _masks
        id0 = consts.tile([128, 128], F16, name="id0")
        _masks.make_identity(nc, id0)
        id_sb = {}
        for l in range(N_LAYERS):
            for di in range(2):
                taps_ld = fw[l] if di == 0 else bw[l]
                for j in range(N_PE_TAPS):
                    it = consts.tile([128, 128], F16, name=f"id{l}{di}{j}")
                    nc.vector.tensor_scalar_mul(it, id0, float(taps_ld[j]))
                    id_sb[(l, di, j)] = it
        bias_sb = consts.tile([128, N_LAYERS * 2 * N_HW * EB], F32, name="bias_sb")
        nc.sync.dma_start(out=bias_sb, in_=bias_dram[:, :])
        pad_sb = consts.tile([128, 2 * PB * W], F16, name="pad_sb")
        nc.scalar.dma_start(out=pad_sb, in_=pad_dram[:, :])
        load_row(1)
        wt_sb = {}
        for l in range(N_LAYERS):
            for h in range(N_HW):
                for di in range(2):
                    for kb in range(PB):
                        wtt = consts.tile([128, 2 * D], F16,
                                          name=f"wt{l}{di}{h}{kb}")
                        eng = nc.sync if di == 0 else nc.scalar
                        eng.dma_start(out=wtt, in_=wt_dram[l, di, h, kb])
                        wt_sb[(l, di, h, kb)] = wtt

        def bias_ap(l, di, h, eb):
            i = ((l * 2 + di) * N_HW + h) * EB + eb
            return bias_sb[:, i:i + 1]

        # layer-1 padded buffers + halos
        xpadf = [bufs.tile([128, BLOC * ROW1], F16, name=f"xpf{blk}")
                 for blk in range(PB)]
        xpadb = [bufs.tile([128, BLOC * ROW1], F16, name=f"xpb{blk}")
                 for blk in range(PB)]
        for blk in range(PB):
            for r in range(BLOC):
                nc.vector.tensor_copy(
                    xpadf[blk][:, r * ROW1:r * ROW1 + W],
                    pad_sb[:, (0 * PB + blk) * W:(0 * PB + blk + 1) * W],
                )
                nc.vector.tensor_copy(
                    xpadb[blk][:, r * ROW1 + S:(r + 1) * ROW1],
                    pad_sb[:, (1 * PB + blk) * W:(1 * PB + blk + 1) * W],
                )

        # activation buffers
        f_t = {di: [bufs.tile([128, T], F16, tag=f"f{di}{blk}",
                              name=f"f{di}{blk}") for blk in range(PB)]
               for di in range(2)}
        xa_t = {di: [bufs.tile([128, T], F16, tag=f"xa{di}{blk}",
                               name=f"xa{di}{blk}") for blk in range(PB)]
                for di in range(2)}
        # layer-1 B outputs alias the (by then dead) conv-output tiles; the
        # per-chunk WAR order (B writes chunk tg after A read it) is tracked
        # by the tile framework.
        xb1 = f_t

        def conv_chunk_dve(l, di, r, dst, src, row_len, base_off, taps):
            """One batch-row conv chunk on DVE: 5 tensor_scalar + 4 adds."""
            for blk in range(PB):
                def sl(j):
                    o = r * row_len + base_off + j
                    return src[blk][:, o:o + CH]
                a = scratch.tile([128, CH], F16, tag="ca", name=f"ca{l}{di}{blk}{r}")
                b = scratch.tile([128, CH], F16, tag="cb", name=f"cb{l}{di}{blk}{r}")
                c = scratch.tile([128, CH], F16, tag="cc", name=f"cc{l}{di}{blk}{r}")
                nc.vector.tensor_scalar_mul(a, sl(0), float(taps[0]))
                nc.vector.tensor_scalar_mul(b, sl(1), float(taps[1]))
                nc.vector.tensor_tensor(a, a, b, AOP.add)
                nc.vector.tensor_scalar_mul(b, sl(2), float(taps[2]))
                nc.vector.tensor_scalar_mul(c, sl(3), float(taps[3]))
                nc.vector.tensor_tensor(b, b, c, AOP.add)
                nc.vector.tensor_scalar_mul(c, sl(4), float(taps[4]))
                nc.vector.tensor_tensor(a, a, b, AOP.add)
                acc = dst[blk][:, r * CH:(r + 1) * CH]
                nc.vector.tensor_tensor(acc, a, c, AOP.add)

        def conv_chunk(l, di, r, dst, src, row_len, base_off, taps):
            """One batch-row conv chunk: CONV_CH sub-chunks."""
            mode = CONV_IMPL.get((l, di), "pe")
            if mode == "dve":
                conv_chunk_dve(l, di, r, dst, src, row_len, base_off, taps)
                return
            n_pe = 3 if mode == "pe3_pool" else N_PE_TAPS
            for c0 in range(0, CH, CONV_CH):
                for blk in range(PB):
                    def sl(j):
                        o = r * row_len + base_off + j + c0
                        return src[blk][:, o:o + CONV_CH]
                    p = psum_cv.tile([128, CONV_CH], F32, tag="cv",
                                     name=f"cv{l}{di}{blk}{r}{c0}")
                    for half in range(CONV_CH // 512):
                        hs = slice(half * 512, (half + 1) * 512)
                        for j in range(n_pe):
                            nc.tensor.matmul(
                                p[:, hs],
                                lhsT=id_sb[(l, di, j)],
                                rhs=sl(j)[:, hs],
                                start=(j == 0),
                                stop=(j == n_pe - 1),
                            )
                    acc = dst[blk][:, r * CH + c0:r * CH + c0 + CONV_CH]
                    if mode == "pe":
                        nc.vector.scalar_tensor_tensor(
                            acc, sl(W), float(taps[W]), p, AOP.mult, AOP.add)
                    elif mode == "pe3_pool":
                        # Pool joins taps 3+4 in SBUF (TS+TS+TT; Pool lacks
                        # the STT opcode); DVE adds the PSUM part
                        tm = scratch.tile([128, CONV_CH], F16, tag="cm",
                                          name=f"cm{l}{di}{blk}{r}{c0}")
                        nc.gpsimd.tensor_scalar_mul(tm, sl(4), float(taps[4]))
                        tm2 = scratch.tile([128, CONV_CH], F16, tag="cn",
                                           name=f"cn{l}{di}{blk}{r}{c0}")
                        nc.gpsimd.tensor_scalar_mul(tm2, sl(3), float(taps[3]))
                        nc.gpsimd.tensor_tensor(tm2, tm2, tm, AOP.add)
                        nc.vector.scalar_tensor_tensor(
                            acc, tm2, 1.0, p, AOP.mult, AOP.add)
                    else:  # pe4_act
                        tm = scratch.tile([128, CONV_CH], F16, tag="cm",
                                          name=f"cm{l}{di}{blk}{r}{c0}")
                        nc.scalar.activation(tm, p, AFT.Copy, bias=0.0,
                                             scale=1.0)
                        tm2 = scratch.tile([128, CONV_CH], F16, tag="cn",
                                           name=f"cn{l}{di}{blk}{r}{c0}")
                        nc.gpsimd.tensor_scalar_mul(tm2, sl(W), float(taps[W]))
                        nc.gpsimd.tensor_tensor(acc, tm2, tm, AOP.add)

        def hw_chunk(l, di, h, tg, x0, x1, x1_row_len, x1_off, c0=0, cw=CH):
            """Token-columns [tg*CH+c0, +cw) of one highway sublayer."""
            gt = {}
            rt = {}
            for half_kind in range(2):      # 0 = nonlin, 1 = gate
                for blk in range(PB):
                    eb = half_kind * PB + blk
                    p = psum_mm.tile([128, cw], F32, tag="mm",
                                     name=f"mm{l}{di}{h}{eb}{tg}{c0}")
                    for h0 in range(0, cw, 512):
                        hw_ = min(512, cw - h0)
                        hs = slice(tg * CH + c0 + h0, tg * CH + c0 + h0 + hw_)
                        for kb in range(PB):
                            nc.tensor.matmul(
                                p[:, h0:h0 + hw_],
                                lhsT=wt_sb[(l, di, h, kb)][:, eb * 128:(eb + 1) * 128],
                                rhs=x0[kb][:, hs],
                                start=(kb == 0),
                                stop=(kb == PB - 1),
                            )
                    if half_kind == 0:
                        t_ = scratch.tile([128, cw], F16, tag="r",
                                          name=f"r{l}{di}{h}{blk}{tg}{c0}")
                        if RELU_DVE_LAST and l == N_LAYERS - 1 and tg == BLOC - 1:
                            nc.vector.tensor_scalar(
                                t_, p, bias_ap(l, di, h, blk), 0.0,
                                AOP.add, AOP.max)
                        else:
                            nc.scalar.activation(
                                t_, p, AFT.Relu, bias=bias_ap(l, di, h, blk),
                                scale=1.0)
                        rt[blk] = t_
                    else:
                        t_ = scratch.tile([128, cw], F16, tag="g",
                                          name=f"g{l}{di}{h}{blk}{tg}{c0}")
                        nc.scalar.activation(
                            t_, p, AFT.Sigmoid,
                            bias=bias_ap(l, di, h, PB + blk), scale=1.0)
                        gt[blk] = t_
            for blk in range(PB):
                x0c = x0[blk][:, tg * CH + c0:tg * CH + c0 + cw]
                tt = scratch.tile([128, cw], F16, tag="t",
                                  name=f"t{l}{di}{h}{blk}{tg}{c0}")
                _eng(nc, COMBINE_ENG.get((l, di, 't'), ENG_DEFAULT['t'])).tensor_tensor(
                    tt, x0c, rt[blk], AOP.subtract)
                ut = scratch.tile([128, cw], F16, tag="u",
                                  name=f"u{l}{di}{h}{blk}{tg}{c0}")
                _eng(nc, COMBINE_ENG.get((l, di, 'u'), ENG_DEFAULT['u'])).tensor_tensor(
                    ut, gt[blk], tt, AOP.mult)
                o = tg * x1_row_len + x1_off + c0
                x1c = x1[blk][:, o:o + cw]
                x1_eng = COMBINE_ENG.get((l, di, 'x1'), ENG_DEFAULT['x1'])
                if l == N_LAYERS - 1 and h == 1 and tg < BLOC - 1:
                    # final-layer B x1 feeds only the output DMA: Pool-able
                    x1_eng = 'p'
                _eng(nc, x1_eng).tensor_tensor(
                    x1c, ut, rt[blk], AOP.add)

        # ---- the network: software-pipelined stage-wise emission ------------
        def src_of(l):
            if l == 0:
                return {0: (xpad0, ROW0, 0), 1: (xpad0, ROW0, W)}
            return {0: (xpadf, ROW1, 0), 1: (xpadb, ROW1, 0)}

        def b_dst_of(l):
            if l == 0:
                return {0: (xpadf, ROW1, W), 1: (xpadb, ROW1, 0)}
            return {0: (xb1[0], CH, 0), 1: (xb1[1], CH, 0)}

        def emit_conv(l, tg):
            for di in range(2):
                taps = fw[l] if di == 0 else bw[l]
                s_tiles, rl, off = src_of(l)[di]
                conv_chunk(l, di, tg, f_t[di], s_tiles, rl, off, taps)

        def emit_A(l, tg, split=False):
            for c0, cw in ([(0, 512), (512, 512)] if split else [(0, CH)]):
                for di in range(2):
                    hw_chunk(l, di, 0, tg, f_t[di], xa_t[di], CH, 0, c0, cw)

        def emit_B(l, tg, split=False):
            for c0, cw in ([(0, 512), (512, 512)] if split else [(0, CH)]):
                for di in range(2):
                    x1t, rl, off = b_dst_of(l)[di]
                    hw_chunk(l, di, 1, tg, xa_t[di], x1t, rl, off, c0, cw)
                    for blk in range(PB):
                        o = tg * rl + off + c0
                        nc.sync.dma_start(
                            out=out[l, di, blk][:, tg * CH + c0:tg * CH + c0 + cw],
                            in_=x1t[blk][:, o:o + cw],
                        )

        # conv chunks interleaved per token group act as PE filler while
        # ACT/DVE drain the highway PSUM groups. Rows 2,3 load lazily so the
        # Pool queue isn't clogged ahead of the first conv evictions.
        for l in range(N_LAYERS):
            for tg in range(BLOC):
                emit_conv(l, tg)
                if l == 0 and tg < 2:
                    load_row(tg + 2)
                last = (l == N_LAYERS - 1 and tg == BLOC - 1)
                emit_A(l, tg)
                emit_B(l, tg, split=last)

        psum_cv.release()
        psum_mm.release()
        scratch.release()
        bufs.release()
        consts.release()

    nc.finalize()
    return nc


def _prep_params(inputs):
    fwd_hw_W = np.asarray(inputs["fwd_hw_W"], np.float32)
    bwd_hw_W = np.asarray(inputs["bwd_hw_W"], np.float32)
    wt = np.empty((N_LAYERS, 2, N_HW, PB, 128, 2 * D), np.float32)
    for l in range(N_LAYERS):
        for di, Wsrc in ((0, fwd_hw_W), (1, bwd_hw_W)):
            for h in range(N_HW):
                wT = Wsrc[l, h].T
                wt[l, di, h] = wT.reshape(PB, 128, 2 * D)
    wt = wt.astype(np.float16)

    fwd_hw_b = np.asarray(inputs["fwd_hw_b"], np.float32)
    bwd_hw_b = np.asarray(inputs["bwd_hw_b"], np.float32)
    bias = np.empty((128, N_LAYERS * 2 * N_HW * EB), np.float32)
    for l in range(N_LAYERS):
        for di, bsrc in ((0, fwd_hw_b), (1, bwd_hw_b)):
            for h in range(N_HW):
                for eb in range(EB):
                    i = ((l * 2 + di) * N_HW + h) * EB + eb
                    bias[:, i] = bsrc[l, h, eb * 128:(eb + 1) * 128]

    fwd_pad = np.asarray(inputs["fwd_pad"], np.float32)
    bwd_pad = np.asarray(inputs["bwd_pad"], np.float32)
    pad1 = np.empty((128, 2 * PB * W), np.float32)
    for di, psrc in ((0, fwd_pad), (1, bwd_pad)):
        pT = psrc[1].T.reshape(PB, 128, W)
        for blk in range(PB):
            pad1[:, (di * PB + blk) * W:(di * PB + blk + 1) * W] = pT[blk]
    pad1 = pad1.astype(np.float16)

    fwd_w = np.asarray(inputs["fwd_w"], np.float32)
    bwd_w = np.asarray(inputs["bwd_w"], np.float32)

    return {
        "wt": np.ascontiguousarray(wt),
        "bias": np.ascontiguousarray(bias),
        "pad1": np.ascontiguousarray(pad1),
        "fwd_w": [[float(v) for v in row] for row in fwd_w],
        "bwd_w": [[float(v) for v in row] for row in bwd_w],
    }


def _prep_core_input(x_core, fwd_pad, bwd_pad):
    xt = np.ascontiguousarray(x_core.transpose(2, 0, 1))
    blocks = xt.reshape(PB, 128, BLOC, S)
    padded = np.empty((PB, 128, BLOC, ROW0), np.float32)
    padded[:, :, :, W:W + S] = blocks
    fr = fwd_pad[0].T.reshape(PB, 128, W)
    bk = bwd_pad[0].T.reshape(PB, 128, W)
    padded[:, :, :, :W] = fr[:, :, None, :]
    padded[:, :, :, W + S:] = bk[:, :, None, :]
    return np.ascontiguousarray(padded.reshape(PB, 128, BLOC * ROW0).astype(np.float16))


_NC_CACHE = {}


def kernel(**inputs):
    params = _prep_params(inputs)
    import hashlib
    h = hashlib.sha256()
    for k in ("wt", "bias", "pad1"):
        h.update(params[k].tobytes())
    h.update(repr(params["fwd_w"]).encode())
    h.update(repr(params["bwd_w"]).encode())
    key = h.hexdigest()
    if key not in _NC_CACHE:
        _NC_CACHE[key] = build_bass(params)
    nc = _NC_CACHE[key]

    x = np.asarray(inputs["inputs"], np.float32)
    fwd_pad = np.asarray(inputs["fwd_pad"], np.float32)
    bwd_pad = np.asarray(inputs["bwd_pad"], np.float32)
    in_maps = [
        {"x": _prep_core_input(x[c * BLOC:(c + 1) * BLOC], fwd_pad, bwd_pad)}
        for c in range(NCORES)
    ]
    res = run_bass_kernel_spmd(nc, in_maps, core_ids=list(range(NCORES)))

    y = np.empty((N_LAYERS, B, S, 2 * D), np.float32)
    for c in range(NCORES):
        o = np.asarray(res.results[c]["out"]).astype(np.float32)
        o = o.reshape(N_LAYERS, 2, PB, 128, BLOC, S)
        o = o.transpose(0, 4, 5, 1, 2, 3).reshape(N_LAYERS, BLOC, S, 2 * D)
        y[:, c * BLOC:(c + 1) * BLOC] = o
    return y
